# revision 17
# baseline (speedup 1.0000x reference)
"""Trainium2 Bass kernel for the ExpertVectorSystem MoE-routing problem.

Reference computation (all fp32):
    we = expert_weights @ expert_vectors              # [B, D]
    for each layer i (8 layers, rank r_i):
        h_i   = relu(we @ w1_i + b1_i)                # [B, 2r]
        out_i = tanh(h_i @ w2_i + b2_i) * 0.1         # [B, r]
    out = concat(out_i, axis=-1)                      # [B, sum(r)]

Strategy: data-parallel over the batch across 8 NeuronCores (2048 rows
each); the tiny per-layer MLP weights are replicated.

Key algebra: we = ew @ v has rank <= 16, so h = relu(ew_aug @ vw1_aug)
with vw1_aug = [[v @ w1], [b1]] ([17, 2r], host-folded).  Stage-1
contraction is K=17 instead of 65, so four chunks pack into the PE's
four 32-row tile groups (tile_position row tiling) and stream the same
moving ew columns concurrently: ~4x fewer stage-1 PE cycles.

All matmuls run in bf16 (fp32 PSUM accumulation): same 1-col/cycle PE
rate as fp32r but half the DMA/SBUF traffic, and bf16 stationaries get
Fast Weight Load so LDWEIGHTS hides completely under the matmul stream.
Simulated end-to-end rel err vs the fp32 reference: 4.3e-3 (fp8 would
be 4.9e-2 - fails the 2e-2 gate, so bf16 is the fastest legal dtype).

Stage-2 is computed transposed: out_pre.T[r, batch] accumulated as
(w2 chunk [128, 128-row-block]) stationary x (hT chunk [128, 512])
moving, so every matmul streams N=512 and every LDWEIGHTS (~96ns with
FWL) hides under the 213ns stream.  b2 rides the tanh activation's
per-partition bias port (free); the *0.1 scale and the final
[r, batch] -> [batch, r] transpose happen on the host.

Per-core schedule: per (layer, 512-col batch group) pair, stage-2 runs
r/128 PSUM accumulation groups (kc matmuls each); the next pair's
stage-1 quads are interleaved one-per-accumulation-group so the relu
drains (alternating ScalarE/VectorE) keep pace and PSUM never backs up.
"""

import contextlib
import ctypes
import os
import sys
import types

import numpy as np
import ml_dtypes

import concourse.bass as bass
import concourse.mybir as mybir
import concourse.tile as tile
from concourse.bass_utils import run_bass_kernel_spmd

B = 16384
E = 16
D = 64
RANKS = [256, 384, 512, 640, 768, 896, 1024, 1152]
STRENGTH = 0.1
NCORES = 8
BL = B // NCORES          # 2048 rows per core
GCOLS = 512               # batch columns per group
NGROUPS = BL // GCOLS     # 4

KC = [2 * r // 128 for r in RANKS]        # stage-2 K chunks per layer
NRB = [r // 128 for r in RANKS]           # output 128-row blocks per layer
QC = [(k + 3) // 4 for k in KC]           # stage-1 quads per layer
QOFF = [sum(QC[:i]) for i in range(len(RANKS))]
RBOFF = [sum(NRB[:i]) for i in range(len(RANKS))]
COLOFF = [sum(RANKS[:i]) for i in range(len(RANKS))]
NQ = sum(QC)              # 24 quad columns in vw1q
NRB_TOT = sum(NRB)        # 46

BF16 = mybir.dt.bfloat16
F32 = mybir.dt.float32
NP_BF16 = ml_dtypes.bfloat16

OUT_COLS = sum(RANKS)     # 5888


def _split_excess_waits(nc):
    """Rewrite instructions carrying >1 sync wait.

    The walrus build in this container accepts at most ONE sync wait per
    instruction ("Too many sync wait commands", CoreV*GenImpl
    setupSyncWait), while Tile's wait assignment freely attaches several.
    Hoist the extra waits onto standalone InstEventSemaphore instructions
    (what BassEngine.wait_ge emits) inserted immediately before the
    instruction on the same engine — same-engine program order makes this
    semantically identical.
    """
    n_split = 0
    for f in nc.m.functions:
        for bb in f.blocks:
            out = []
            dirty = False
            for ins in bb.instructions:
                si = ins.sync_info
                waits = list(si.on_wait) if si is not None else []
                if len(waits) > 1:
                    dirty = True
                    for k, w in enumerate(waits[:-1]):
                        out.append(
                            mybir.InstEventSemaphore(
                                name=f"{ins.name}_xw{k}",
                                engine=ins.engine,
                                ins=[],
                                outs=[],
                                sync_info=mybir.SyncInfo(
                                    on_wait=[w], on_update=[]
                                ),
                            )
                        )
                        n_split += 1
                    ins.sync_info = mybir.SyncInfo(
                        on_wait=[waits[-1]], on_update=list(si.on_update)
                    )
                out.append(ins)
            if dirty:
                bb.instructions = out
    return n_split


def _build_program():
    nc = bass.Bass()
    vw1q_d = nc.declare_dram_parameter("vw1q", [128, NQ * 128], BF16,
                                       isOutput=False)
    ewr_d = nc.declare_dram_parameter("ewr", [128, BL], BF16, isOutput=False)
    b2_d = nc.declare_dram_parameter("b2blk", [128, NRB_TOT], F32,
                                     isOutput=False)
    w2_d = [
        nc.declare_dram_parameter(f"w2_{i}", [128, KC[i] * RANKS[i]], BF16,
                                  isOutput=False)
        for i in range(len(RANKS))
    ]
    outT_d = nc.declare_dram_parameter("outT", [OUT_COLS, BL], BF16,
                                       isOutput=True)

    with tile.TileContext(nc) as tc:
        with (
            tc.tile_pool(name="const", bufs=1) as cpool,
            tc.tile_pool(name="hpsum", bufs=6, space="PSUM") as hpsum,
            tc.tile_pool(name="opsum", bufs=2, space="PSUM") as opsum,
            tc.tile_pool(name="w2", bufs=1) as w2pool,
            tc.tile_pool(name="h", bufs=2) as hpool,
            tc.tile_pool(name="osb", bufs=6) as osb,
        ):
            # ---- startup: PE warm-up on a memset tile + sliced DMAs ----
            # Warm-up needs no input data (memset), so it starts at ~0 and
            # runs in the same (32,128) tile mode as the stage-1 quads: the
            # HAM clock gate reaches 8/8 (2.4 GHz) while the first DMAs
            # stream and no mode-switch drain precedes the first real quad.
            # a ~6us framework preamble (engine barrier + const loads) runs
            # before any user instruction, so only a short warm bridge is
            # needed until the first input slices land (~1.5us later).
            wz = cpool.tile([32, 128], BF16, name="warm_zeros")
            nc.vector.memset(wz[:], 0.0)
            for k in range(12):
                warm = opsum.tile([128, 64], F32, tag="op", name=f"warm_{k}")
                nc.tensor.matmul(
                    warm[:], wz[0:17, 0:128], wz[0:17, 0:64],
                    start=True, stop=True, tile_position=(0, 0),
                )

            # first (layer0, group0) slices land first so real work can
            # begin ~2us in; the bulk loads stream behind them.
            vw1q_sb = cpool.tile([128, NQ * 128], BF16, name="vw1q_sb")
            nc.sync.dma_start(vw1q_sb[:, 0:128], vw1q_d[:, 0:128])
            ewr_sb = cpool.tile([128, BL], BF16, name="ewr_sb")
            nc.sync.dma_start(ewr_sb[:, 0:GCOLS], ewr_d[:, 0:GCOLS])
            b2_sb = cpool.tile([128, NRB_TOT], F32, name="b2_sb")

            # layers are processed small/large interleaved so the small
            # layers' drain-heavy, compute-light pairs hide under the big
            # layers' long stage-2 windows
            layer_order = [0, 4, 1, 5, 2, 6, 3, 7]
            w2_fam = {li: i % 4 for i, li in enumerate(layer_order)}

            def load_w2(li):
                r = RANKS[li]
                tiles = []
                for c in range(KC[li]):
                    # 4 rotating tag families: a layer's DMAs only wait on
                    # reads of the layer 4 processing-slots back (long
                    # done), so they stream pairs ahead instead of
                    # stalling on the current layer's final reads.
                    t = w2pool.tile([128, r], BF16,
                                    tag=f"w2_{w2_fam[li]}_{c}",
                                    name=f"w2_{li}_{c}")
                    nc.sync.dma_start(t[:], w2_d[li][:, c * r:(c + 1) * r])
                    tiles.append(t)
                return tiles

            w2_sb = {0: load_w2(0)}
            w2_sb[4] = load_w2(4)
            nc.sync.dma_start(b2_sb[:], b2_d[:])
            nc.sync.dma_start(vw1q_sb[:, 128:NQ * 128],
                              vw1q_d[:, 128:NQ * 128])
            nc.sync.dma_start(ewr_sb[:, GCOLS:BL], ewr_d[:, GCOLS:BL])

            # ---- stage 1: h chunks via 4-packed 32-row-tile matmuls ----
            def stage1_quads(li, g, h_sb, act_cols=256):
                """Yield thunks; each emits one quad of K=17 matmuls into
                the PE's four 32-row tile groups (concurrent on HW) plus
                their relu drains split across ScalarE/VectorE.  act_cols
                sets ScalarE's share of each drain (it also runs the tanh,
                so small-kc host pairs give it a lighter slice)."""
                qo = QOFF[li]
                for q in range(QC[li]):
                    def unit(q=q):
                        nt = min(4, KC[li] - 4 * q)
                        hps = []
                        for t in range(nt):
                            hp = hpsum.tile([128, GCOLS], F32, tag="hp",
                                            name=f"hp_{li}_{g}_{4*q+t}")
                            nc.tensor.matmul(
                                hp[:],
                                vw1q_sb[32 * t:32 * t + 17,
                                        (qo + q) * 128:(qo + q + 1) * 128],
                                ewr_sb[32 * t:32 * t + 17,
                                       g * GCOLS:(g + 1) * GCOLS],
                                start=True, stop=True,
                                tile_position=(32 * t, 0),
                            )
                            hps.append(hp)
                        for t, hp in enumerate(hps):
                            c = 4 * q + t
                            ht = hpool.tile([128, GCOLS], BF16, tag=f"h_{c}",
                                            name=f"h_{li}_{g}_{c}")
                            # split each relu drain across both engines so
                            # the hp PSUM bank recycles fast and the next
                            # quad never stalls on bank availability
                            nc.scalar.activation(
                                ht[:, 0:act_cols], hp[:, 0:act_cols],
                                mybir.ActivationFunctionType.Relu,
                            )
                            nc.vector.tensor_scalar_max(
                                ht[:, act_cols:GCOLS], hp[:, act_cols:GCOLS],
                                0.0,
                            )
                            h_sb.append(ht)
                    yield unit

            # ---- main sweep over (layer, batch-group) pairs ----
            pairs = []
            for ci in range(0, len(layer_order), 2):
                a, b = layer_order[ci], layer_order[ci + 1]
                for g in range(NGROUPS):
                    pairs.append((a, g))
                    pairs.append((b, g))
            loaded = {0, 4}
            h_cur = []
            for u in stage1_quads(0, 0, h_cur, act_cols=128):
                u()
            for idx, (li, g) in enumerate(pairs):
                r = RANKS[li]
                kc = KC[li]
                nrb = NRB[li]
                # prefetch w2 three pairs ahead so even the biggest layer's
                # DMA (~15us) completes before its first use
                for f in (1, 2, 3):
                    if idx + f < len(pairs):
                        fli = pairs[idx + f][0]
                        if fli not in loaded:
                            loaded.add(fli)
                            w2_sb[fli] = load_w2(fli)
                nxt = pairs[idx + 1] if idx + 1 < len(pairs) else None
                h_nxt = []
                units = []
                if nxt is not None:
                    nli, ng = nxt
                    units = list(stage1_quads(
                        nli, ng, h_nxt,
                        act_cols=(128 if kc <= 8 else 256),
                    ))
                ui = 0
                for rb in range(nrb):
                    op = opsum.tile([128, GCOLS], F32, tag="op",
                                    name=f"op_{li}_{g}_{rb}")
                    for c in range(kc):
                        nc.tensor.matmul(
                            op[:],
                            w2_sb[li][c][:, rb * 128:(rb + 1) * 128],
                            h_cur[c][:],
                            start=(c == 0), stop=(c == kc - 1),
                        )
                    ot = osb.tile([128, GCOLS], BF16, tag="ot",
                                  name=f"ot_{li}_{g}_{rb}")
                    rbg = RBOFF[li] + rb
                    nc.scalar.activation(
                        ot[:], op[:], mybir.ActivationFunctionType.Tanh,
                        bias=b2_sb[:, rbg:rbg + 1],
                    )
                    row0 = COLOFF[li] + rb * 128
                    nc.sync.dma_start(
                        outT_d[row0:row0 + 128, g * GCOLS:(g + 1) * GCOLS],
                        ot[:],
                    )
                    # next pair's stage-1 quads spread across accumulation
                    # groups (at most 2 per group when a small layer hosts
                    # a big one): the relu drains keep clearing hp banks
                    # while the next group's matmuls stream.
                    target = ((rb + 1) * len(units) + nrb - 1) // nrb
                    while ui < min(target, len(units)):
                        units[ui]()
                        ui += 1
                for u in units[ui:]:
                    u()
                h_cur = h_nxt
    _split_excess_waits(nc)
    return nc


_CACHE = {}


def _get_program():
    if "p" not in _CACHE:
        _CACHE["p"] = _build_program()
    return _CACHE["p"]


def _prepare_inputs(inputs):
    """Host-side marshalling: fold v@w1+b1 into the quad-packed stage-1
    stationary, chunk w2, build the replicated [ew^T; ones] bands."""
    ew = np.asarray(inputs["expert_weights"], dtype=np.float32)
    v = np.asarray(inputs["expert_vectors"], dtype=np.float32)

    vw1q = np.zeros((128, NQ * 128), np.float32)
    b2blk = np.zeros((128, NRB_TOT), np.float32)
    w2cat = []
    for i, r in enumerate(RANKS):
        w1 = np.asarray(inputs[f"w1_{i}"], dtype=np.float32)   # [D, 2r]
        b1 = np.asarray(inputs[f"b1_{i}"], dtype=np.float32)   # [2r]
        w2 = np.asarray(inputs[f"w2_{i}"], dtype=np.float32)   # [2r, r]
        b2 = np.asarray(inputs[f"b2_{i}"], dtype=np.float32)   # [r]
        vw1a = np.concatenate([v @ w1, b1[None, :]], axis=0)   # [17, 2r]
        for c in range(KC[i]):
            q, t = divmod(c, 4)
            vw1q[32 * t:32 * t + 17,
                 (QOFF[i] + q) * 128:(QOFF[i] + q + 1) * 128] = \
                vw1a[:, c * 128:(c + 1) * 128]
        w2cat.append(np.ascontiguousarray(
            w2.reshape(KC[i], 128, r).transpose(1, 0, 2).reshape(128, -1)
        ).astype(NP_BF16))
        b2blk[:, RBOFF[i]:RBOFF[i] + NRB[i]] = b2.reshape(NRB[i], 128).T
    vw1q = vw1q.astype(NP_BF16)

    ewT1 = np.concatenate([ew.T, np.ones((1, B), np.float32)], axis=0)

    in_maps = []
    for core in range(NCORES):
        er = np.zeros((128, BL), np.float32)
        sl = ewT1[:, core * BL:(core + 1) * BL]
        for t in range(4):
            er[32 * t:32 * t + 17] = sl
        m = {
            "vw1q": vw1q,
            "ewr": er.astype(NP_BF16),
            "b2blk": b2blk,
        }
        for i in range(len(RANKS)):
            m[f"w2_{i}"] = w2cat[i]
        in_maps.append(m)
    return in_maps


def _install_ntff_hook():
    """Provide antenv.axon_hooks if the image lacks it (trace support).

    run_bass_kernel_spmd's axon trace path imports
    antenv.axon_hooks.get_axon_ntff_profile_hook; this container's antenv
    has no such module, so recreate the ctypes-based hook against the
    injected libaxon_pjrt.so (same as trn_agent_boot._ntff_profile_via_ctypes).
    """
    try:
        from antenv.axon_hooks import get_axon_ntff_profile_hook  # noqa: F401
        return
    except ImportError:
        pass
    so_path = "/opt/axon/libaxon_pjrt.so"
    hook = None
    if os.path.exists(so_path):
        lib = ctypes.CDLL(so_path)
        if hasattr(lib, "axon_start_nrt_profile"):
            lib.axon_start_nrt_profile.argtypes = [
                ctypes.POINTER(ctypes.c_int64),
                ctypes.c_size_t,
            ]
            lib.axon_start_nrt_profile.restype = ctypes.c_int64
            lib.axon_stop_nrt_profile.argtypes = [ctypes.c_char_p]
            lib.axon_stop_nrt_profile.restype = ctypes.c_int64

            @contextlib.contextmanager
            def _hook(output_dir, device_ids):
                import jax

                jax.devices()
                if device_ids:
                    ids = (ctypes.c_int64 * len(device_ids))(*device_ids)
                    rc = lib.axon_start_nrt_profile(ids, len(device_ids))
                else:
                    rc = lib.axon_start_nrt_profile(None, 0)
                if rc != 0:
                    raise RuntimeError(f"axon_start_nrt_profile rc={rc}")
                try:
                    yield
                finally:
                    n = lib.axon_stop_nrt_profile(str(output_dir).encode())
                    if n < 0:
                        raise RuntimeError(f"axon_stop_nrt_profile rc={n}")

            hook = _hook

    import antenv

    mod = types.ModuleType("antenv.axon_hooks")
    state = {"hook": hook}
    mod.get_axon_ntff_profile_hook = lambda: state["hook"]
    mod.set_axon_ntff_profile_hook = lambda h: state.__setitem__("hook", h)
    sys.modules["antenv.axon_hooks"] = mod
    antenv.axon_hooks = mod


def run(inputs, trace=False, tmpdir=None):
    """Run the kernel on all 8 cores; returns (full_output, BassKernelResults)."""
    if trace:
        _install_ntff_hook()
    nc = _get_program()
    in_maps = _prepare_inputs(inputs)
    res = run_bass_kernel_spmd(
        nc, in_maps, core_ids=list(range(NCORES)), trace=trace, tmpdir=tmpdir
    )
    # device emits tanh(x)+... transposed [OUT_COLS, BL] in bf16; the *0.1
    # scale and the transpose back to [BL, OUT_COLS] happen here.
    parts = []
    for i in range(NCORES):
        o = res.results[i]["outT"].astype(np.float32)
        parts.append(o.T * np.float32(STRENGTH))
    out = np.ascontiguousarray(np.concatenate(parts, axis=0),
                               dtype=np.float32)
    return out, res


def kernel(**inputs) -> np.ndarray:
    out, _ = run(inputs, trace=False)
    return out


# revision 21
# speedup vs baseline: 1.0157x; 1.0157x over previous
"""Trainium2 Bass kernel for the ExpertVectorSystem MoE-routing problem.

Reference computation (all fp32):
    we = expert_weights @ expert_vectors              # [B, D]
    for each layer i (8 layers, rank r_i):
        h_i   = relu(we @ w1_i + b1_i)                # [B, 2r]
        out_i = tanh(h_i @ w2_i + b2_i) * 0.1         # [B, r]
    out = concat(out_i, axis=-1)                      # [B, sum(r)]

Strategy: data-parallel over the batch across 8 NeuronCores (2048 rows
each); the tiny per-layer MLP weights are replicated.

Key algebra: we = ew @ v has rank <= 16, so h = relu(ew_aug @ vw1_aug)
with vw1_aug = [[v @ w1], [b1]] ([17, 2r], host-folded).  Stage-1
contraction is K=17 instead of 65, so four chunks pack into the PE's
four 32-row tile groups (tile_position row tiling) and stream the same
moving ew columns concurrently: ~4x fewer stage-1 PE cycles.

All matmuls run in bf16 (fp32 PSUM accumulation): same 1-col/cycle PE
rate as fp32r but half the DMA/SBUF traffic, and bf16 stationaries get
Fast Weight Load so LDWEIGHTS hides completely under the matmul stream.
Simulated end-to-end rel err vs the fp32 reference: 4.3e-3 (fp8 would
be 4.9e-2 - fails the 2e-2 gate, so bf16 is the fastest legal dtype).

Stage-2 is computed transposed: out_pre.T[r, batch] accumulated as
(w2 chunk [128, 128-row-block]) stationary x (hT chunk [128, 512])
moving, so every matmul streams N=512 and every LDWEIGHTS (~96ns with
FWL) hides under the 213ns stream.  b2 rides the tanh activation's
per-partition bias port (free); the *0.1 scale and the final
[r, batch] -> [batch, r] transpose happen on the host.

Per-core schedule: per (layer, 512-col batch group) pair, stage-2 runs
r/128 PSUM accumulation groups (kc matmuls each); the next pair's
stage-1 quads are interleaved one-per-accumulation-group so the relu
drains (alternating ScalarE/VectorE) keep pace and PSUM never backs up.
"""

import contextlib
import ctypes
import os
import sys
import types

import numpy as np
import ml_dtypes

import concourse.bass as bass
import concourse.mybir as mybir
import concourse.tile as tile
from concourse.bass_utils import run_bass_kernel_spmd

B = 16384
E = 16
D = 64
RANKS = [256, 384, 512, 640, 768, 896, 1024, 1152]
STRENGTH = 0.1
NCORES = 8
BL = B // NCORES          # 2048 rows per core
GCOLS = 512               # batch columns per group
NGROUPS = BL // GCOLS     # 4

KC = [2 * r // 128 for r in RANKS]        # stage-2 K chunks per layer
NRB = [r // 128 for r in RANKS]           # output 128-row blocks per layer
QC = [(k + 3) // 4 for k in KC]           # stage-1 quads per layer
QOFF = [sum(QC[:i]) for i in range(len(RANKS))]
RBOFF = [sum(NRB[:i]) for i in range(len(RANKS))]
COLOFF = [sum(RANKS[:i]) for i in range(len(RANKS))]
NQ = sum(QC)              # 24 quad columns in vw1q
NRB_TOT = sum(NRB)        # 46

BF16 = mybir.dt.bfloat16
F32 = mybir.dt.float32
NP_BF16 = ml_dtypes.bfloat16

OUT_COLS = sum(RANKS)     # 5888


def _split_excess_waits(nc):
    """Rewrite instructions carrying >1 sync wait.

    The walrus build in this container accepts at most ONE sync wait per
    instruction ("Too many sync wait commands", CoreV*GenImpl
    setupSyncWait), while Tile's wait assignment freely attaches several.
    Hoist the extra waits onto standalone InstEventSemaphore instructions
    (what BassEngine.wait_ge emits) inserted immediately before the
    instruction on the same engine — same-engine program order makes this
    semantically identical.
    """
    n_split = 0
    for f in nc.m.functions:
        for bb in f.blocks:
            out = []
            dirty = False
            for ins in bb.instructions:
                si = ins.sync_info
                waits = list(si.on_wait) if si is not None else []
                if len(waits) > 1:
                    dirty = True
                    for k, w in enumerate(waits[:-1]):
                        out.append(
                            mybir.InstEventSemaphore(
                                name=f"{ins.name}_xw{k}",
                                engine=ins.engine,
                                ins=[],
                                outs=[],
                                sync_info=mybir.SyncInfo(
                                    on_wait=[w], on_update=[]
                                ),
                            )
                        )
                        n_split += 1
                    ins.sync_info = mybir.SyncInfo(
                        on_wait=[waits[-1]], on_update=list(si.on_update)
                    )
                out.append(ins)
            if dirty:
                bb.instructions = out
    return n_split


def _build_program():
    nc = bass.Bass()
    vw1q_d = nc.declare_dram_parameter("vw1q", [128, NQ * 128], BF16,
                                       isOutput=False)
    ewr_d = nc.declare_dram_parameter("ewr", [128, BL], BF16, isOutput=False)
    b2_d = nc.declare_dram_parameter("b2blk", [128, NRB_TOT], F32,
                                     isOutput=False)
    w2_d = [
        nc.declare_dram_parameter(f"w2_{i}", [128, KC[i] * RANKS[i]], BF16,
                                  isOutput=False)
        for i in range(len(RANKS))
    ]
    outT_d = nc.declare_dram_parameter("outT", [OUT_COLS, BL], BF16,
                                       isOutput=True)

    with tile.TileContext(nc) as tc:
        with (
            tc.tile_pool(name="const", bufs=1) as cpool,
            tc.tile_pool(name="hpsum", bufs=6, space="PSUM") as hpsum,
            tc.tile_pool(name="opsum", bufs=2, space="PSUM") as opsum,
            tc.tile_pool(name="w2", bufs=1) as w2pool,
            tc.tile_pool(name="h", bufs=2) as hpool,
            tc.tile_pool(name="osb", bufs=6) as osb,
        ):
            # ---- startup: PE warm-up on a memset tile + sliced DMAs ----
            # Warm-up needs no input data (memset), so it starts at ~0 and
            # runs in the same (32,128) tile mode as the stage-1 quads: the
            # HAM clock gate reaches 8/8 (2.4 GHz) while the first DMAs
            # stream and no mode-switch drain precedes the first real quad.
            # a ~6us framework preamble (engine barrier + const loads) runs
            # before any user instruction, so only a short warm bridge is
            # needed until the first input slices land (~1.5us later).
            wz = cpool.tile([32, 128], BF16, name="warm_zeros")
            nc.vector.memset(wz[:], 0.0)
            for k in range(12):
                warm = opsum.tile([128, 64], F32, tag="op", name=f"warm_{k}")
                nc.tensor.matmul(
                    warm[:], wz[0:17, 0:128], wz[0:17, 0:64],
                    start=True, stop=True, tile_position=(0, 0),
                )

            # first (layer0, group0) slices land first so real work can
            # begin ~2us in; the bulk loads stream behind them on a
            # different queue.
            vw1q_sb = cpool.tile([128, NQ * 128], BF16, name="vw1q_sb")
            nc.scalar.dma_start(vw1q_sb[:, 0:128], vw1q_d[:, 0:128])
            ewr_sb = cpool.tile([128, BL], BF16, name="ewr_sb")
            nc.scalar.dma_start(ewr_sb[:, 0:GCOLS], ewr_d[:, 0:GCOLS])
            b2_sb = cpool.tile([128, NRB_TOT], F32, name="b2_sb")

            # layers are processed small/large interleaved so the small
            # layers' drain-heavy, compute-light pairs hide under the big
            # layers' long stage-2 windows
            layer_order = [0, 4, 1, 5, 2, 6, 3, 7]
            w2_fam = {li: i % 4 for i, li in enumerate(layer_order)}

            # input DMAs ride different engines' queues (the engine only
            # writes a doorbell) so the big weight streams run on parallel
            # DMA rings instead of serializing behind one queue; outputs
            # keep the sync queue to themselves.
            w2_q = {0: nc.gpsimd, 1: nc.gpsimd, 2: nc.scalar, 3: nc.scalar}

            def load_w2(li):
                r = RANKS[li]
                tiles = []
                eng = w2_q[w2_fam[li]]
                for c in range(KC[li]):
                    # 4 rotating tag families: a layer's DMAs only wait on
                    # reads of the layer 4 processing-slots back (long
                    # done), so they stream pairs ahead instead of
                    # stalling on the current layer's final reads.
                    t = w2pool.tile([128, r], BF16,
                                    tag=f"w2_{w2_fam[li]}_{c}",
                                    name=f"w2_{li}_{c}")
                    eng.dma_start(t[:], w2_d[li][:, c * r:(c + 1) * r])
                    tiles.append(t)
                return tiles

            w2_sb = {0: load_w2(0)}
            w2_sb[4] = load_w2(4)
            nc.scalar.dma_start(b2_sb[:], b2_d[:])
            nc.scalar.dma_start(vw1q_sb[:, 128:NQ * 128],
                                vw1q_d[:, 128:NQ * 128])
            nc.scalar.dma_start(ewr_sb[:, GCOLS:BL], ewr_d[:, GCOLS:BL])

            # ---- stage 1: h chunks via 4-packed 32-row-tile matmuls ----
            def stage1_quads(li, g, h_sb, act_cols=256):
                """Yield thunks; each emits one quad of K=17 matmuls into
                the PE's four 32-row tile groups (concurrent on HW) plus
                their relu drains split across ScalarE/VectorE.  act_cols
                sets ScalarE's share of each drain (it also runs the tanh,
                so small-kc host pairs give it a lighter slice)."""
                qo = QOFF[li]
                for q in range(QC[li]):
                    def unit(q=q):
                        nt = min(4, KC[li] - 4 * q)
                        hps = []
                        for t in range(nt):
                            hp = hpsum.tile([128, GCOLS], F32, tag="hp",
                                            name=f"hp_{li}_{g}_{4*q+t}")
                            nc.tensor.matmul(
                                hp[:],
                                vw1q_sb[32 * t:32 * t + 17,
                                        (qo + q) * 128:(qo + q + 1) * 128],
                                ewr_sb[32 * t:32 * t + 17,
                                       g * GCOLS:(g + 1) * GCOLS],
                                start=True, stop=True,
                                tile_position=(32 * t, 0),
                            )
                            hps.append(hp)
                        for t, hp in enumerate(hps):
                            c = 4 * q + t
                            ht = hpool.tile([128, GCOLS], BF16, tag=f"h_{c}",
                                            name=f"h_{li}_{g}_{c}")
                            # split each relu drain across both engines so
                            # the hp PSUM bank recycles fast and the next
                            # quad never stalls on bank availability
                            nc.scalar.activation(
                                ht[:, 0:act_cols], hp[:, 0:act_cols],
                                mybir.ActivationFunctionType.Relu,
                            )
                            nc.vector.tensor_scalar_max(
                                ht[:, act_cols:GCOLS], hp[:, act_cols:GCOLS],
                                0.0,
                            )
                            h_sb.append(ht)
                    yield unit

            # ---- main sweep over (layer, batch-group) pairs ----
            pairs = []
            for ci in range(0, len(layer_order), 2):
                a, b = layer_order[ci], layer_order[ci + 1]
                for g in range(NGROUPS):
                    pairs.append((a, g))
                    pairs.append((b, g))
            loaded = {0, 4}
            h_cur = []
            for u in stage1_quads(0, 0, h_cur, act_cols=128):
                u()
            for idx, (li, g) in enumerate(pairs):
                r = RANKS[li]
                kc = KC[li]
                nrb = NRB[li]
                # prefetch w2 three pairs ahead so even the biggest layer's
                # DMA (~15us) completes before its first use
                for f in (1, 2, 3):
                    if idx + f < len(pairs):
                        fli = pairs[idx + f][0]
                        if fli not in loaded:
                            loaded.add(fli)
                            w2_sb[fli] = load_w2(fli)
                nxt = pairs[idx + 1] if idx + 1 < len(pairs) else None
                h_nxt = []
                units = []
                if nxt is not None:
                    nli, ng = nxt
                    units = list(stage1_quads(
                        nli, ng, h_nxt,
                        act_cols=(128 if kc <= 8 else 256),
                    ))
                ui = 0
                for rb in range(nrb):
                    op = opsum.tile([128, GCOLS], F32, tag="op",
                                    name=f"op_{li}_{g}_{rb}")
                    for c in range(kc):
                        nc.tensor.matmul(
                            op[:],
                            w2_sb[li][c][:, rb * 128:(rb + 1) * 128],
                            h_cur[c][:],
                            start=(c == 0), stop=(c == kc - 1),
                        )
                    ot = osb.tile([128, GCOLS], BF16, tag="ot",
                                  name=f"ot_{li}_{g}_{rb}")
                    rbg = RBOFF[li] + rb
                    nc.scalar.activation(
                        ot[:], op[:], mybir.ActivationFunctionType.Tanh,
                        bias=b2_sb[:, rbg:rbg + 1],
                    )
                    row0 = COLOFF[li] + rb * 128
                    nc.sync.dma_start(
                        outT_d[row0:row0 + 128, g * GCOLS:(g + 1) * GCOLS],
                        ot[:],
                    )
                    # next pair's stage-1 quads spread across accumulation
                    # groups (at most 2 per group when a small layer hosts
                    # a big one): the relu drains keep clearing hp banks
                    # while the next group's matmuls stream.
                    target = ((rb + 1) * len(units) + nrb - 1) // nrb
                    while ui < min(target, len(units)):
                        units[ui]()
                        ui += 1
                for u in units[ui:]:
                    u()
                h_cur = h_nxt
    _split_excess_waits(nc)
    return nc


_CACHE = {}


def _get_program():
    if "p" not in _CACHE:
        _CACHE["p"] = _build_program()
    return _CACHE["p"]


def _prepare_inputs(inputs):
    """Host-side marshalling: fold v@w1+b1 into the quad-packed stage-1
    stationary, chunk w2, build the replicated [ew^T; ones] bands."""
    ew = np.asarray(inputs["expert_weights"], dtype=np.float32)
    v = np.asarray(inputs["expert_vectors"], dtype=np.float32)

    vw1q = np.zeros((128, NQ * 128), np.float32)
    b2blk = np.zeros((128, NRB_TOT), np.float32)
    w2cat = []
    for i, r in enumerate(RANKS):
        w1 = np.asarray(inputs[f"w1_{i}"], dtype=np.float32)   # [D, 2r]
        b1 = np.asarray(inputs[f"b1_{i}"], dtype=np.float32)   # [2r]
        w2 = np.asarray(inputs[f"w2_{i}"], dtype=np.float32)   # [2r, r]
        b2 = np.asarray(inputs[f"b2_{i}"], dtype=np.float32)   # [r]
        vw1a = np.concatenate([v @ w1, b1[None, :]], axis=0)   # [17, 2r]
        for c in range(KC[i]):
            q, t = divmod(c, 4)
            vw1q[32 * t:32 * t + 17,
                 (QOFF[i] + q) * 128:(QOFF[i] + q + 1) * 128] = \
                vw1a[:, c * 128:(c + 1) * 128]
        w2cat.append(np.ascontiguousarray(
            w2.reshape(KC[i], 128, r).transpose(1, 0, 2).reshape(128, -1)
        ).astype(NP_BF16))
        b2blk[:, RBOFF[i]:RBOFF[i] + NRB[i]] = b2.reshape(NRB[i], 128).T
    vw1q = vw1q.astype(NP_BF16)

    ewT1 = np.concatenate([ew.T, np.ones((1, B), np.float32)], axis=0)

    in_maps = []
    for core in range(NCORES):
        er = np.zeros((128, BL), np.float32)
        sl = ewT1[:, core * BL:(core + 1) * BL]
        for t in range(4):
            er[32 * t:32 * t + 17] = sl
        m = {
            "vw1q": vw1q,
            "ewr": er.astype(NP_BF16),
            "b2blk": b2blk,
        }
        for i in range(len(RANKS)):
            m[f"w2_{i}"] = w2cat[i]
        in_maps.append(m)
    return in_maps


def _install_ntff_hook():
    """Provide antenv.axon_hooks if the image lacks it (trace support).

    run_bass_kernel_spmd's axon trace path imports
    antenv.axon_hooks.get_axon_ntff_profile_hook; this container's antenv
    has no such module, so recreate the ctypes-based hook against the
    injected libaxon_pjrt.so (same as trn_agent_boot._ntff_profile_via_ctypes).
    """
    try:
        from antenv.axon_hooks import get_axon_ntff_profile_hook  # noqa: F401
        return
    except ImportError:
        pass
    so_path = "/opt/axon/libaxon_pjrt.so"
    hook = None
    if os.path.exists(so_path):
        lib = ctypes.CDLL(so_path)
        if hasattr(lib, "axon_start_nrt_profile"):
            lib.axon_start_nrt_profile.argtypes = [
                ctypes.POINTER(ctypes.c_int64),
                ctypes.c_size_t,
            ]
            lib.axon_start_nrt_profile.restype = ctypes.c_int64
            lib.axon_stop_nrt_profile.argtypes = [ctypes.c_char_p]
            lib.axon_stop_nrt_profile.restype = ctypes.c_int64

            @contextlib.contextmanager
            def _hook(output_dir, device_ids):
                import jax

                jax.devices()
                if device_ids:
                    ids = (ctypes.c_int64 * len(device_ids))(*device_ids)
                    rc = lib.axon_start_nrt_profile(ids, len(device_ids))
                else:
                    rc = lib.axon_start_nrt_profile(None, 0)
                if rc != 0:
                    raise RuntimeError(f"axon_start_nrt_profile rc={rc}")
                try:
                    yield
                finally:
                    n = lib.axon_stop_nrt_profile(str(output_dir).encode())
                    if n < 0:
                        raise RuntimeError(f"axon_stop_nrt_profile rc={n}")

            hook = _hook

    import antenv

    mod = types.ModuleType("antenv.axon_hooks")
    state = {"hook": hook}
    mod.get_axon_ntff_profile_hook = lambda: state["hook"]
    mod.set_axon_ntff_profile_hook = lambda h: state.__setitem__("hook", h)
    sys.modules["antenv.axon_hooks"] = mod
    antenv.axon_hooks = mod


def run(inputs, trace=False, tmpdir=None):
    """Run the kernel on all 8 cores; returns (full_output, BassKernelResults)."""
    if trace:
        _install_ntff_hook()
    nc = _get_program()
    in_maps = _prepare_inputs(inputs)
    res = run_bass_kernel_spmd(
        nc, in_maps, core_ids=list(range(NCORES)), trace=trace, tmpdir=tmpdir
    )
    # device emits tanh(x)+... transposed [OUT_COLS, BL] in bf16; the *0.1
    # scale and the transpose back to [BL, OUT_COLS] happen here.
    parts = []
    for i in range(NCORES):
        o = res.results[i]["outT"].astype(np.float32)
        parts.append(o.T * np.float32(STRENGTH))
    out = np.ascontiguousarray(np.concatenate(parts, axis=0),
                               dtype=np.float32)
    return out, res


def kernel(**inputs) -> np.ndarray:
    out, _ = run(inputs, trace=False)
    return out


# revision 24
# speedup vs baseline: 1.0707x; 1.0542x over previous
"""Trainium2 Bass kernel for the ExpertVectorSystem MoE-routing problem.

Reference computation (all fp32):
    we = expert_weights @ expert_vectors              # [B, D]
    for each layer i (8 layers, rank r_i):
        h_i   = relu(we @ w1_i + b1_i)                # [B, 2r]
        out_i = tanh(h_i @ w2_i + b2_i) * 0.1         # [B, r]
    out = concat(out_i, axis=-1)                      # [B, sum(r)]

Strategy: data-parallel over the batch across 8 NeuronCores (2048 rows
each); the tiny per-layer MLP weights are replicated.

Key algebra: we = ew @ v has rank <= 16, so h = relu(ew_aug @ vw1_aug)
with vw1_aug = [[v @ w1], [b1]] ([17, 2r], host-folded).  Stage-1
contraction is K=17 instead of 65, so four chunks pack into the PE's
four 32-row tile groups (tile_position row tiling) and stream the same
moving ew columns concurrently: ~4x fewer stage-1 PE cycles.

All matmuls run in bf16 (fp32 PSUM accumulation): same 1-col/cycle PE
rate as fp32r but half the DMA/SBUF traffic, and bf16 stationaries get
Fast Weight Load so LDWEIGHTS hides completely under the matmul stream.
Simulated end-to-end rel err vs the fp32 reference: 4.3e-3 (fp8 would
be 4.9e-2 - fails the 2e-2 gate, so bf16 is the fastest legal dtype).

Stage-2 is computed transposed: out_pre.T[r, batch] accumulated as
(w2 chunk [128, 128-row-block]) stationary x (hT chunk [128, 512])
moving, so every matmul streams N=512 and every LDWEIGHTS (~96ns with
FWL) hides under the 213ns stream.  b2 rides the tanh activation's
per-partition bias port (free); the *0.1 scale and the final
[r, batch] -> [batch, r] transpose happen on the host.

Per-core schedule: per (layer, 512-col batch group) pair, stage-2 runs
r/128 PSUM accumulation groups (kc matmuls each); the next pair's
stage-1 quads are interleaved one-per-accumulation-group so the relu
drains (alternating ScalarE/VectorE) keep pace and PSUM never backs up.
"""

import contextlib
import ctypes
import os
import sys
import types

import numpy as np
import ml_dtypes

import concourse.bass as bass
import concourse.mybir as mybir
import concourse.tile as tile
from concourse.bass_utils import run_bass_kernel_spmd

B = 16384
E = 16
D = 64
RANKS = [256, 384, 512, 640, 768, 896, 1024, 1152]
STRENGTH = 0.1
NCORES = 8
BL = B // NCORES          # 2048 rows per core
GCOLS = 512               # batch columns per group
NGROUPS = BL // GCOLS     # 4

KC = [2 * r // 128 for r in RANKS]        # stage-2 K chunks per layer
NRB = [r // 128 for r in RANKS]           # output 128-row blocks per layer
QC = [(k + 3) // 4 for k in KC]           # stage-1 quads per layer
QOFF = [sum(QC[:i]) for i in range(len(RANKS))]
RBOFF = [sum(NRB[:i]) for i in range(len(RANKS))]
COLOFF = [sum(RANKS[:i]) for i in range(len(RANKS))]
NQ = sum(QC)              # 24 quad columns in vw1q
NRB_TOT = sum(NRB)        # 46

BF16 = mybir.dt.bfloat16
F32 = mybir.dt.float32
NP_BF16 = ml_dtypes.bfloat16

OUT_COLS = sum(RANKS)     # 5888


def _split_excess_waits(nc):
    """Rewrite instructions carrying >1 sync wait.

    The walrus build in this container accepts at most ONE sync wait per
    instruction ("Too many sync wait commands", CoreV*GenImpl
    setupSyncWait), while Tile's wait assignment freely attaches several.
    Hoist the extra waits onto standalone InstEventSemaphore instructions
    (what BassEngine.wait_ge emits) inserted immediately before the
    instruction on the same engine — same-engine program order makes this
    semantically identical.
    """
    n_split = 0
    for f in nc.m.functions:
        for bb in f.blocks:
            out = []
            dirty = False
            for ins in bb.instructions:
                si = ins.sync_info
                waits = list(si.on_wait) if si is not None else []
                if len(waits) > 1:
                    dirty = True
                    for k, w in enumerate(waits[:-1]):
                        out.append(
                            mybir.InstEventSemaphore(
                                name=f"{ins.name}_xw{k}",
                                engine=ins.engine,
                                ins=[],
                                outs=[],
                                sync_info=mybir.SyncInfo(
                                    on_wait=[w], on_update=[]
                                ),
                            )
                        )
                        n_split += 1
                    ins.sync_info = mybir.SyncInfo(
                        on_wait=[waits[-1]], on_update=list(si.on_update)
                    )
                out.append(ins)
            if dirty:
                bb.instructions = out
    return n_split


def _build_program():
    nc = bass.Bass()
    vw1q_d = nc.declare_dram_parameter("vw1q", [128, NQ * 128], BF16,
                                       isOutput=False)
    ewr_d = nc.declare_dram_parameter("ewr", [128, BL], BF16, isOutput=False)
    b2_d = nc.declare_dram_parameter("b2blk", [128, NRB_TOT], F32,
                                     isOutput=False)
    w2_d = [
        nc.declare_dram_parameter(f"w2_{i}", [128, KC[i] * RANKS[i]], BF16,
                                  isOutput=False)
        for i in range(len(RANKS))
    ]
    outT_d = nc.declare_dram_parameter("outT", [OUT_COLS, BL], BF16,
                                       isOutput=True)

    with tile.TileContext(nc) as tc:
        with (
            tc.tile_pool(name="const", bufs=1) as cpool,
            tc.tile_pool(name="hpsum", bufs=6, space="PSUM") as hpsum,
            tc.tile_pool(name="opsum", bufs=2, space="PSUM") as opsum,
            tc.tile_pool(name="w2", bufs=1) as w2pool,
            tc.tile_pool(name="h", bufs=3) as hpool,
            tc.tile_pool(name="osb", bufs=6) as osb,
        ):
            # ---- startup: PE warm-up on a memset tile + sliced DMAs ----
            # Warm-up needs no input data (memset), so it starts at ~0 and
            # runs in the same (32,128) tile mode as the stage-1 quads: the
            # HAM clock gate reaches 8/8 (2.4 GHz) while the first DMAs
            # stream and no mode-switch drain precedes the first real quad.
            # a ~6us framework preamble (engine barrier + const loads) runs
            # before any user instruction, so only a short warm bridge is
            # needed until the first input slices land (~1.5us later).
            wz = cpool.tile([32, 128], BF16, name="warm_zeros")
            nc.vector.memset(wz[:], 0.0)
            for k in range(12):
                warm = opsum.tile([128, 64], F32, tag="op", name=f"warm_{k}")
                nc.tensor.matmul(
                    warm[:], wz[0:17, 0:128], wz[0:17, 0:64],
                    start=True, stop=True, tile_position=(0, 0),
                )

            # first (layer0, group0) slices land first so real work can
            # begin ~2us in; the bulk loads stream behind them on a
            # different queue.
            vw1q_sb = cpool.tile([128, NQ * 128], BF16, name="vw1q_sb")
            nc.scalar.dma_start(vw1q_sb[:, 0:128], vw1q_d[:, 0:128])
            ewr_sb = cpool.tile([128, BL], BF16, name="ewr_sb")
            nc.scalar.dma_start(ewr_sb[:, 0:GCOLS], ewr_d[:, 0:GCOLS])
            b2_sb = cpool.tile([128, NRB_TOT], F32, name="b2_sb")

            # layers are processed small/large interleaved so the small
            # layers' drain-heavy, compute-light pairs hide under the big
            # layers' long stage-2 windows
            layer_order = [0, 4, 1, 5, 2, 6, 3, 7]
            w2_fam = {li: i % 4 for i, li in enumerate(layer_order)}

            # input DMAs ride different engines' queues (the engine only
            # writes a doorbell) so the big weight streams run on parallel
            # DMA rings instead of serializing behind one queue; outputs
            # keep the sync queue to themselves.
            w2_q = {0: nc.gpsimd, 1: nc.gpsimd, 2: nc.scalar, 3: nc.scalar}

            def load_w2(li):
                r = RANKS[li]
                tiles = []
                eng = w2_q[w2_fam[li]]
                for c in range(KC[li]):
                    # 4 rotating tag families: a layer's DMAs only wait on
                    # reads of the layer 4 processing-slots back (long
                    # done), so they stream pairs ahead instead of
                    # stalling on the current layer's final reads.
                    t = w2pool.tile([128, r], BF16,
                                    tag=f"w2_{w2_fam[li]}_{c}",
                                    name=f"w2_{li}_{c}")
                    eng.dma_start(t[:], w2_d[li][:, c * r:(c + 1) * r])
                    tiles.append(t)
                return tiles

            w2_sb = {0: load_w2(0)}
            w2_sb[4] = load_w2(4)
            nc.scalar.dma_start(b2_sb[:], b2_d[:])
            nc.scalar.dma_start(vw1q_sb[:, 128:NQ * 128],
                                vw1q_d[:, 128:NQ * 128])
            nc.scalar.dma_start(ewr_sb[:, GCOLS:BL], ewr_d[:, GCOLS:BL])

            # ---- stage 1: h chunks via 4-packed 32-row-tile matmuls ----
            def stage1_quads(li, g, h_sb, act_cols=256):
                """Yield thunks; each emits one quad of K=17 matmuls into
                the PE's four 32-row tile groups (concurrent on HW) plus
                their relu drains split across ScalarE/VectorE.  act_cols
                sets ScalarE's share of each drain (it also runs the tanh,
                so small-kc host pairs give it a lighter slice)."""
                qo = QOFF[li]
                for q in range(QC[li]):
                    def unit(q=q):
                        nt = min(4, KC[li] - 4 * q)
                        hps = []
                        for t in range(nt):
                            hp = hpsum.tile([128, GCOLS], F32, tag="hp",
                                            name=f"hp_{li}_{g}_{4*q+t}")
                            nc.tensor.matmul(
                                hp[:],
                                vw1q_sb[32 * t:32 * t + 17,
                                        (qo + q) * 128:(qo + q + 1) * 128],
                                ewr_sb[32 * t:32 * t + 17,
                                       g * GCOLS:(g + 1) * GCOLS],
                                start=True, stop=True,
                                tile_position=(32 * t, 0),
                            )
                            hps.append(hp)
                        for t, hp in enumerate(hps):
                            c = 4 * q + t
                            ht = hpool.tile([128, GCOLS], BF16, tag=f"h_{c}",
                                            name=f"h_{li}_{g}_{c}")
                            # split each relu drain across both engines so
                            # the hp PSUM bank recycles fast and the next
                            # quad never stalls on bank availability
                            nc.scalar.activation(
                                ht[:, 0:act_cols], hp[:, 0:act_cols],
                                mybir.ActivationFunctionType.Relu,
                            )
                            nc.vector.tensor_scalar_max(
                                ht[:, act_cols:GCOLS], hp[:, act_cols:GCOLS],
                                0.0,
                            )
                            h_sb.append(ht)
                    yield unit

            # ---- main sweep over (layer, batch-group) pairs ----
            pairs = []
            for ci in range(0, len(layer_order), 2):
                a, b = layer_order[ci], layer_order[ci + 1]
                for g in range(NGROUPS):
                    pairs.append((a, g))
                    pairs.append((b, g))
            # stage-1 production is hosted only by BIG pairs (their long
            # stage-2 windows absorb the relu-drain bursts): each big pair
            # produces h for the next small AND next big pair.  Small
            # pairs host nothing (their windows are drain-starved).
            is_big = [NRB[p[0]] >= 6 for p in pairs]
            producer = {}
            for j in range(1, len(pairs)):
                if is_big[j - 1] or j < 2:
                    producer[j] = j - 1
                elif is_big[j - 2]:
                    producer[j] = j - 2
                else:
                    producer[j] = j - 1
            hosted = {}
            for j, p in producer.items():
                hosted.setdefault(p, []).append(j)

            loaded = {0, 4}
            h_lists = {0: []}
            for u in stage1_quads(0, 0, h_lists[0], act_cols=128):
                u()
            for idx, (li, g) in enumerate(pairs):
                r = RANKS[li]
                kc = KC[li]
                nrb = NRB[li]
                # prefetch w2 three pairs ahead so even the biggest layer's
                # DMA (~15us) completes before its first use
                for f in (1, 2, 3):
                    if idx + f < len(pairs):
                        fli = pairs[idx + f][0]
                        if fli not in loaded:
                            loaded.add(fli)
                            w2_sb[fli] = load_w2(fli)
                h_cur = h_lists.pop(idx)
                units = []
                for j in sorted(hosted.get(idx, [])):
                    jli, jg = pairs[j]
                    h_lists[j] = []
                    units.extend(stage1_quads(
                        jli, jg, h_lists[j],
                        act_cols=(128 if kc <= 8 else 256),
                    ))
                ui = 0
                for rb in range(nrb):
                    op = opsum.tile([128, GCOLS], F32, tag="op",
                                    name=f"op_{li}_{g}_{rb}")
                    for c in range(kc):
                        nc.tensor.matmul(
                            op[:],
                            w2_sb[li][c][:, rb * 128:(rb + 1) * 128],
                            h_cur[c][:],
                            start=(c == 0), stop=(c == kc - 1),
                        )
                    ot = osb.tile([128, GCOLS], BF16, tag="ot",
                                  name=f"ot_{li}_{g}_{rb}")
                    rbg = RBOFF[li] + rb
                    nc.scalar.activation(
                        ot[:], op[:], mybir.ActivationFunctionType.Tanh,
                        bias=b2_sb[:, rbg:rbg + 1],
                    )
                    row0 = COLOFF[li] + rb * 128
                    nc.sync.dma_start(
                        outT_d[row0:row0 + 128, g * GCOLS:(g + 1) * GCOLS],
                        ot[:],
                    )
                    # next pair's stage-1 quads spread across accumulation
                    # groups (at most 2 per group when a small layer hosts
                    # a big one): the relu drains keep clearing hp banks
                    # while the next group's matmuls stream.
                    target = ((rb + 1) * len(units) + nrb - 1) // nrb
                    while ui < min(target, len(units)):
                        units[ui]()
                        ui += 1
                for u in units[ui:]:
                    u()
    _split_excess_waits(nc)
    return nc


_CACHE = {}


def _get_program():
    if "p" not in _CACHE:
        _CACHE["p"] = _build_program()
    return _CACHE["p"]


def _prepare_inputs(inputs):
    """Host-side marshalling: fold v@w1+b1 into the quad-packed stage-1
    stationary, chunk w2, build the replicated [ew^T; ones] bands."""
    ew = np.asarray(inputs["expert_weights"], dtype=np.float32)
    v = np.asarray(inputs["expert_vectors"], dtype=np.float32)

    vw1q = np.zeros((128, NQ * 128), np.float32)
    b2blk = np.zeros((128, NRB_TOT), np.float32)
    w2cat = []
    for i, r in enumerate(RANKS):
        w1 = np.asarray(inputs[f"w1_{i}"], dtype=np.float32)   # [D, 2r]
        b1 = np.asarray(inputs[f"b1_{i}"], dtype=np.float32)   # [2r]
        w2 = np.asarray(inputs[f"w2_{i}"], dtype=np.float32)   # [2r, r]
        b2 = np.asarray(inputs[f"b2_{i}"], dtype=np.float32)   # [r]
        vw1a = np.concatenate([v @ w1, b1[None, :]], axis=0)   # [17, 2r]
        for c in range(KC[i]):
            q, t = divmod(c, 4)
            vw1q[32 * t:32 * t + 17,
                 (QOFF[i] + q) * 128:(QOFF[i] + q + 1) * 128] = \
                vw1a[:, c * 128:(c + 1) * 128]
        w2cat.append(np.ascontiguousarray(
            w2.reshape(KC[i], 128, r).transpose(1, 0, 2).reshape(128, -1)
        ).astype(NP_BF16))
        b2blk[:, RBOFF[i]:RBOFF[i] + NRB[i]] = b2.reshape(NRB[i], 128).T
    vw1q = vw1q.astype(NP_BF16)

    ewT1 = np.concatenate([ew.T, np.ones((1, B), np.float32)], axis=0)

    in_maps = []
    for core in range(NCORES):
        er = np.zeros((128, BL), np.float32)
        sl = ewT1[:, core * BL:(core + 1) * BL]
        for t in range(4):
            er[32 * t:32 * t + 17] = sl
        m = {
            "vw1q": vw1q,
            "ewr": er.astype(NP_BF16),
            "b2blk": b2blk,
        }
        for i in range(len(RANKS)):
            m[f"w2_{i}"] = w2cat[i]
        in_maps.append(m)
    return in_maps


def _install_ntff_hook():
    """Provide antenv.axon_hooks if the image lacks it (trace support).

    run_bass_kernel_spmd's axon trace path imports
    antenv.axon_hooks.get_axon_ntff_profile_hook; this container's antenv
    has no such module, so recreate the ctypes-based hook against the
    injected libaxon_pjrt.so (same as trn_agent_boot._ntff_profile_via_ctypes).
    """
    try:
        from antenv.axon_hooks import get_axon_ntff_profile_hook  # noqa: F401
        return
    except ImportError:
        pass
    so_path = "/opt/axon/libaxon_pjrt.so"
    hook = None
    if os.path.exists(so_path):
        lib = ctypes.CDLL(so_path)
        if hasattr(lib, "axon_start_nrt_profile"):
            lib.axon_start_nrt_profile.argtypes = [
                ctypes.POINTER(ctypes.c_int64),
                ctypes.c_size_t,
            ]
            lib.axon_start_nrt_profile.restype = ctypes.c_int64
            lib.axon_stop_nrt_profile.argtypes = [ctypes.c_char_p]
            lib.axon_stop_nrt_profile.restype = ctypes.c_int64

            @contextlib.contextmanager
            def _hook(output_dir, device_ids):
                import jax

                jax.devices()
                if device_ids:
                    ids = (ctypes.c_int64 * len(device_ids))(*device_ids)
                    rc = lib.axon_start_nrt_profile(ids, len(device_ids))
                else:
                    rc = lib.axon_start_nrt_profile(None, 0)
                if rc != 0:
                    raise RuntimeError(f"axon_start_nrt_profile rc={rc}")
                try:
                    yield
                finally:
                    n = lib.axon_stop_nrt_profile(str(output_dir).encode())
                    if n < 0:
                        raise RuntimeError(f"axon_stop_nrt_profile rc={n}")

            hook = _hook

    import antenv

    mod = types.ModuleType("antenv.axon_hooks")
    state = {"hook": hook}
    mod.get_axon_ntff_profile_hook = lambda: state["hook"]
    mod.set_axon_ntff_profile_hook = lambda h: state.__setitem__("hook", h)
    sys.modules["antenv.axon_hooks"] = mod
    antenv.axon_hooks = mod


def run(inputs, trace=False, tmpdir=None):
    """Run the kernel on all 8 cores; returns (full_output, BassKernelResults)."""
    if trace:
        _install_ntff_hook()
    nc = _get_program()
    in_maps = _prepare_inputs(inputs)
    res = run_bass_kernel_spmd(
        nc, in_maps, core_ids=list(range(NCORES)), trace=trace, tmpdir=tmpdir
    )
    # device emits tanh(x)+... transposed [OUT_COLS, BL] in bf16; the *0.1
    # scale and the transpose back to [BL, OUT_COLS] happen here.
    parts = []
    for i in range(NCORES):
        o = res.results[i]["outT"].astype(np.float32)
        parts.append(o.T * np.float32(STRENGTH))
    out = np.ascontiguousarray(np.concatenate(parts, axis=0),
                               dtype=np.float32)
    return out, res


def kernel(**inputs) -> np.ndarray:
    out, _ = run(inputs, trace=False)
    return out


# revision 28
# speedup vs baseline: 1.0737x; 1.0028x over previous
"""Trainium2 Bass kernel for the ExpertVectorSystem MoE-routing problem.

Reference computation (all fp32):
    we = expert_weights @ expert_vectors              # [B, D]
    for each layer i (8 layers, rank r_i):
        h_i   = relu(we @ w1_i + b1_i)                # [B, 2r]
        out_i = tanh(h_i @ w2_i + b2_i) * 0.1         # [B, r]
    out = concat(out_i, axis=-1)                      # [B, sum(r)]

Strategy: data-parallel over the batch across 8 NeuronCores (2048 rows
each); the tiny per-layer MLP weights are replicated.

Key algebra: we = ew @ v has rank <= 16, so h = relu(ew_aug @ vw1_aug)
with vw1_aug = [[v @ w1], [b1]] ([17, 2r], host-folded).  Stage-1
contraction is K=17 instead of 65, so four chunks pack into the PE's
four 32-row tile groups (tile_position row tiling) and stream the same
moving ew columns concurrently: ~4x fewer stage-1 PE cycles.

All matmuls run in bf16 (fp32 PSUM accumulation): same 1-col/cycle PE
rate as fp32r but half the DMA/SBUF traffic, and bf16 stationaries get
Fast Weight Load so LDWEIGHTS hides completely under the matmul stream.
Simulated end-to-end rel err vs the fp32 reference: 4.3e-3 (fp8 would
be 4.9e-2 - fails the 2e-2 gate, so bf16 is the fastest legal dtype).

Stage-2 is computed transposed: out_pre.T[r, batch] accumulated as
(w2 chunk [128, 128-row-block]) stationary x (hT chunk [128, 512])
moving, so every matmul streams N=512 and every LDWEIGHTS (~96ns with
FWL) hides under the 213ns stream.  b2 rides the tanh activation's
per-partition bias port (free); the *0.1 scale and the final
[r, batch] -> [batch, r] transpose happen on the host.

Per-core schedule: per (layer, 512-col batch group) pair, stage-2 runs
r/128 PSUM accumulation groups (kc matmuls each); the next pair's
stage-1 quads are interleaved one-per-accumulation-group so the relu
drains (alternating ScalarE/VectorE) keep pace and PSUM never backs up.
"""

import contextlib
import ctypes
import os
import sys
import types

import numpy as np
import ml_dtypes

import concourse.bass as bass
import concourse.mybir as mybir
import concourse.tile as tile
from concourse.bass_utils import run_bass_kernel_spmd

B = 16384
E = 16
D = 64
RANKS = [256, 384, 512, 640, 768, 896, 1024, 1152]
STRENGTH = 0.1
NCORES = 8
BL = B // NCORES          # 2048 rows per core
GCOLS = 512               # batch columns per group
NGROUPS = BL // GCOLS     # 4

KC = [2 * r // 128 for r in RANKS]        # stage-2 K chunks per layer
NRB = [r // 128 for r in RANKS]           # output 128-row blocks per layer
QC = [(k + 3) // 4 for k in KC]           # stage-1 quads per layer
QOFF = [sum(QC[:i]) for i in range(len(RANKS))]
RBOFF = [sum(NRB[:i]) for i in range(len(RANKS))]
COLOFF = [sum(RANKS[:i]) for i in range(len(RANKS))]
NQ = sum(QC)              # 24 quad columns in vw1q
NRB_TOT = sum(NRB)        # 46

BF16 = mybir.dt.bfloat16
F32 = mybir.dt.float32
NP_BF16 = ml_dtypes.bfloat16

OUT_COLS = sum(RANKS)     # 5888


def _split_excess_waits(nc):
    """Rewrite instructions carrying >1 sync wait.

    The walrus build in this container accepts at most ONE sync wait per
    instruction ("Too many sync wait commands", CoreV*GenImpl
    setupSyncWait), while Tile's wait assignment freely attaches several.
    Hoist the extra waits onto standalone InstEventSemaphore instructions
    (what BassEngine.wait_ge emits) inserted immediately before the
    instruction on the same engine — same-engine program order makes this
    semantically identical.
    """
    n_split = 0
    for f in nc.m.functions:
        for bb in f.blocks:
            out = []
            dirty = False
            for ins in bb.instructions:
                si = ins.sync_info
                waits = list(si.on_wait) if si is not None else []
                if len(waits) > 1:
                    dirty = True
                    for k, w in enumerate(waits[:-1]):
                        out.append(
                            mybir.InstEventSemaphore(
                                name=f"{ins.name}_xw{k}",
                                engine=ins.engine,
                                ins=[],
                                outs=[],
                                sync_info=mybir.SyncInfo(
                                    on_wait=[w], on_update=[]
                                ),
                            )
                        )
                        n_split += 1
                    ins.sync_info = mybir.SyncInfo(
                        on_wait=[waits[-1]], on_update=list(si.on_update)
                    )
                out.append(ins)
            if dirty:
                bb.instructions = out
    return n_split


def _build_program():
    nc = bass.Bass()
    vw1q_d = nc.declare_dram_parameter("vw1q", [128, NQ * 128], BF16,
                                       isOutput=False)
    ewr_d = nc.declare_dram_parameter("ewr", [128, BL], BF16, isOutput=False)
    b2_d = nc.declare_dram_parameter("b2blk", [128, NRB_TOT], F32,
                                     isOutput=False)
    w2_d = [
        nc.declare_dram_parameter(f"w2_{i}", [128, KC[i] * RANKS[i]], BF16,
                                  isOutput=False)
        for i in range(len(RANKS))
    ]
    outT_d = nc.declare_dram_parameter("outT", [OUT_COLS, BL], BF16,
                                       isOutput=True)

    with tile.TileContext(nc) as tc:
        with (
            tc.tile_pool(name="const", bufs=1) as cpool,
            tc.tile_pool(name="hpsum", bufs=6, space="PSUM") as hpsum,
            tc.tile_pool(name="opsum", bufs=2, space="PSUM") as opsum,
            tc.tile_pool(name="w2", bufs=1) as w2pool,
            tc.tile_pool(name="h", bufs=3) as hpool,
            tc.tile_pool(name="osb", bufs=6) as osb,
        ):
            # ---- startup: PE warm-up on a memset tile + sliced DMAs ----
            # Warm-up needs no input data (memset), so it starts at ~0 and
            # runs in the same (32,128) tile mode as the stage-1 quads: the
            # HAM clock gate reaches 8/8 (2.4 GHz) while the first DMAs
            # stream and no mode-switch drain precedes the first real quad.
            # a ~6us framework preamble (engine barrier + const loads) runs
            # before any user instruction, so only a short warm bridge is
            # needed until the first input slices land (~1.5us later).
            wz = cpool.tile([32, 128], BF16, name="warm_zeros")
            nc.vector.memset(wz[:], 0.0)
            for k in range(12):
                warm = opsum.tile([128, 64], F32, tag="op", name=f"warm_{k}")
                nc.tensor.matmul(
                    warm[:], wz[0:17, 0:128], wz[0:17, 0:64],
                    start=True, stop=True, tile_position=(0, 0),
                )

            # first (layer0, group0) slices land first so real work can
            # begin ~2us in; the bulk loads stream behind them on a
            # different queue.
            vw1q_sb = cpool.tile([128, NQ * 128], BF16, name="vw1q_sb")
            nc.scalar.dma_start(vw1q_sb[:, 0:128], vw1q_d[:, 0:128])
            ewr_sb = cpool.tile([128, BL], BF16, name="ewr_sb")
            nc.scalar.dma_start(ewr_sb[:, 0:GCOLS], ewr_d[:, 0:GCOLS])
            b2_sb = cpool.tile([128, NRB_TOT], F32, name="b2_sb")

            # layers processed in rank order (measured faster than
            # small/large interleaving, which concentrates h-buffer and
            # PSUM lifetimes without paying for itself)
            layer_order = list(range(len(RANKS)))
            w2_fam = {li: i % 4 for i, li in enumerate(layer_order)}

            # input DMAs ride different engines' queues (the engine only
            # writes a doorbell) so the big weight streams run on parallel
            # DMA rings instead of serializing behind one queue; outputs
            # keep the sync queue to themselves.
            w2_q = {0: nc.gpsimd, 1: nc.gpsimd, 2: nc.scalar, 3: nc.scalar}

            def load_w2(li):
                r = RANKS[li]
                tiles = []
                eng = w2_q[w2_fam[li]]
                for c in range(KC[li]):
                    # 4 rotating tag families: a layer's DMAs only wait on
                    # reads of the layer 4 processing-slots back (long
                    # done), so they stream pairs ahead instead of
                    # stalling on the current layer's final reads.
                    t = w2pool.tile([128, r], BF16,
                                    tag=f"w2_{w2_fam[li]}_{c}",
                                    name=f"w2_{li}_{c}")
                    eng.dma_start(t[:], w2_d[li][:, c * r:(c + 1) * r])
                    tiles.append(t)
                return tiles

            w2_sb = {0: load_w2(0)}
            nc.scalar.dma_start(b2_sb[:], b2_d[:])
            nc.scalar.dma_start(vw1q_sb[:, 128:NQ * 128],
                                vw1q_d[:, 128:NQ * 128])
            nc.scalar.dma_start(ewr_sb[:, GCOLS:BL], ewr_d[:, GCOLS:BL])

            # ---- stage 1: h chunks via 4-packed 32-row-tile matmuls ----
            def stage1_quads(li, g, h_sb, act_cols=256):
                """Yield thunks; each emits one quad of K=17 matmuls into
                the PE's four 32-row tile groups (concurrent on HW) plus
                their relu drains split across ScalarE/VectorE.  act_cols
                sets ScalarE's share of each drain (it also runs the tanh,
                so small-kc host pairs give it a lighter slice)."""
                qo = QOFF[li]
                for q in range(QC[li]):
                    def unit(q=q):
                        nt = min(4, KC[li] - 4 * q)
                        hps = []
                        for t in range(nt):
                            hp = hpsum.tile([128, GCOLS], F32, tag="hp",
                                            name=f"hp_{li}_{g}_{4*q+t}")
                            nc.tensor.matmul(
                                hp[:],
                                vw1q_sb[32 * t:32 * t + 17,
                                        (qo + q) * 128:(qo + q + 1) * 128],
                                ewr_sb[32 * t:32 * t + 17,
                                       g * GCOLS:(g + 1) * GCOLS],
                                start=True, stop=True,
                                tile_position=(32 * t, 0),
                            )
                            hps.append(hp)
                        for t, hp in enumerate(hps):
                            c = 4 * q + t
                            ht = hpool.tile([128, GCOLS], BF16, tag=f"h_{c}",
                                            name=f"h_{li}_{g}_{c}")
                            # split each relu drain across both engines so
                            # the hp PSUM bank recycles fast and the next
                            # quad never stalls on bank availability
                            nc.scalar.activation(
                                ht[:, 0:act_cols], hp[:, 0:act_cols],
                                mybir.ActivationFunctionType.Relu,
                            )
                            nc.vector.tensor_scalar_max(
                                ht[:, act_cols:GCOLS], hp[:, act_cols:GCOLS],
                                0.0,
                            )
                            h_sb.append(ht)
                    yield unit

            # ---- main sweep over (layer, batch-group) pairs ----
            pairs = [(li, g) for li in layer_order for g in range(NGROUPS)]
            # stage-1 production is hosted only by BIG pairs (their long
            # stage-2 windows absorb the relu-drain bursts): each big pair
            # produces h for the next small AND next big pair.  Small
            # pairs host nothing (their windows are drain-starved).
            is_big = [NRB[p[0]] >= 6 for p in pairs]
            producer = {}
            for j in range(1, len(pairs)):
                if is_big[j - 1] or j < 2:
                    producer[j] = j - 1
                elif is_big[j - 2]:
                    producer[j] = j - 2
                else:
                    producer[j] = j - 1
            hosted = {}
            for j, p in producer.items():
                hosted.setdefault(p, []).append(j)

            loaded = {0}
            h_lists = {0: []}
            for u in stage1_quads(0, 0, h_lists[0], act_cols=128):
                u()
            for idx, (li, g) in enumerate(pairs):
                r = RANKS[li]
                kc = KC[li]
                nrb = NRB[li]
                # prefetch w2 three pairs ahead so even the biggest layer's
                # DMA (~15us) completes before its first use
                for f in (1, 2, 3):
                    if idx + f < len(pairs):
                        fli = pairs[idx + f][0]
                        if fli not in loaded:
                            loaded.add(fli)
                            w2_sb[fli] = load_w2(fli)
                h_cur = h_lists.pop(idx)
                units = []
                for j in sorted(hosted.get(idx, [])):
                    jli, jg = pairs[j]
                    h_lists[j] = []
                    units.extend(stage1_quads(
                        jli, jg, h_lists[j],
                        act_cols=(128 if kc <= 8 else 256),
                    ))
                ui = 0
                for rb in range(nrb):
                    op = opsum.tile([128, GCOLS], F32, tag="op",
                                    name=f"op_{li}_{g}_{rb}")
                    for c in range(kc):
                        nc.tensor.matmul(
                            op[:],
                            w2_sb[li][c][:, rb * 128:(rb + 1) * 128],
                            h_cur[c][:],
                            start=(c == 0), stop=(c == kc - 1),
                        )
                    ot = osb.tile([128, GCOLS], BF16, tag="ot",
                                  name=f"ot_{li}_{g}_{rb}")
                    rbg = RBOFF[li] + rb
                    nc.scalar.activation(
                        ot[:], op[:], mybir.ActivationFunctionType.Tanh,
                        bias=b2_sb[:, rbg:rbg + 1],
                    )
                    row0 = COLOFF[li] + rb * 128
                    nc.sync.dma_start(
                        outT_d[row0:row0 + 128, g * GCOLS:(g + 1) * GCOLS],
                        ot[:],
                    )
                    # next pair's stage-1 quads spread across accumulation
                    # groups (at most 2 per group when a small layer hosts
                    # a big one): the relu drains keep clearing hp banks
                    # while the next group's matmuls stream.
                    target = ((rb + 1) * len(units) + nrb - 1) // nrb
                    while ui < min(target, len(units)):
                        units[ui]()
                        ui += 1
                for u in units[ui:]:
                    u()
    _split_excess_waits(nc)
    return nc


_CACHE = {}


def _get_program():
    if "p" not in _CACHE:
        _CACHE["p"] = _build_program()
    return _CACHE["p"]


def _prepare_inputs(inputs):
    """Host-side marshalling: fold v@w1+b1 into the quad-packed stage-1
    stationary, chunk w2, build the replicated [ew^T; ones] bands."""
    ew = np.asarray(inputs["expert_weights"], dtype=np.float32)
    v = np.asarray(inputs["expert_vectors"], dtype=np.float32)

    vw1q = np.zeros((128, NQ * 128), np.float32)
    b2blk = np.zeros((128, NRB_TOT), np.float32)
    w2cat = []
    for i, r in enumerate(RANKS):
        w1 = np.asarray(inputs[f"w1_{i}"], dtype=np.float32)   # [D, 2r]
        b1 = np.asarray(inputs[f"b1_{i}"], dtype=np.float32)   # [2r]
        w2 = np.asarray(inputs[f"w2_{i}"], dtype=np.float32)   # [2r, r]
        b2 = np.asarray(inputs[f"b2_{i}"], dtype=np.float32)   # [r]
        vw1a = np.concatenate([v @ w1, b1[None, :]], axis=0)   # [17, 2r]
        for c in range(KC[i]):
            q, t = divmod(c, 4)
            vw1q[32 * t:32 * t + 17,
                 (QOFF[i] + q) * 128:(QOFF[i] + q + 1) * 128] = \
                vw1a[:, c * 128:(c + 1) * 128]
        w2cat.append(np.ascontiguousarray(
            w2.reshape(KC[i], 128, r).transpose(1, 0, 2).reshape(128, -1)
        ).astype(NP_BF16))
        b2blk[:, RBOFF[i]:RBOFF[i] + NRB[i]] = b2.reshape(NRB[i], 128).T
    vw1q = vw1q.astype(NP_BF16)

    ewT1 = np.concatenate([ew.T, np.ones((1, B), np.float32)], axis=0)

    in_maps = []
    for core in range(NCORES):
        er = np.zeros((128, BL), np.float32)
        sl = ewT1[:, core * BL:(core + 1) * BL]
        for t in range(4):
            er[32 * t:32 * t + 17] = sl
        m = {
            "vw1q": vw1q,
            "ewr": er.astype(NP_BF16),
            "b2blk": b2blk,
        }
        for i in range(len(RANKS)):
            m[f"w2_{i}"] = w2cat[i]
        in_maps.append(m)
    return in_maps


def _install_ntff_hook():
    """Provide antenv.axon_hooks if the image lacks it (trace support).

    run_bass_kernel_spmd's axon trace path imports
    antenv.axon_hooks.get_axon_ntff_profile_hook; this container's antenv
    has no such module, so recreate the ctypes-based hook against the
    injected libaxon_pjrt.so (same as trn_agent_boot._ntff_profile_via_ctypes).
    """
    try:
        from antenv.axon_hooks import get_axon_ntff_profile_hook  # noqa: F401
        return
    except ImportError:
        pass
    so_path = "/opt/axon/libaxon_pjrt.so"
    hook = None
    if os.path.exists(so_path):
        lib = ctypes.CDLL(so_path)
        if hasattr(lib, "axon_start_nrt_profile"):
            lib.axon_start_nrt_profile.argtypes = [
                ctypes.POINTER(ctypes.c_int64),
                ctypes.c_size_t,
            ]
            lib.axon_start_nrt_profile.restype = ctypes.c_int64
            lib.axon_stop_nrt_profile.argtypes = [ctypes.c_char_p]
            lib.axon_stop_nrt_profile.restype = ctypes.c_int64

            @contextlib.contextmanager
            def _hook(output_dir, device_ids):
                import jax

                jax.devices()
                if device_ids:
                    ids = (ctypes.c_int64 * len(device_ids))(*device_ids)
                    rc = lib.axon_start_nrt_profile(ids, len(device_ids))
                else:
                    rc = lib.axon_start_nrt_profile(None, 0)
                if rc != 0:
                    raise RuntimeError(f"axon_start_nrt_profile rc={rc}")
                try:
                    yield
                finally:
                    n = lib.axon_stop_nrt_profile(str(output_dir).encode())
                    if n < 0:
                        raise RuntimeError(f"axon_stop_nrt_profile rc={n}")

            hook = _hook

    import antenv

    mod = types.ModuleType("antenv.axon_hooks")
    state = {"hook": hook}
    mod.get_axon_ntff_profile_hook = lambda: state["hook"]
    mod.set_axon_ntff_profile_hook = lambda h: state.__setitem__("hook", h)
    sys.modules["antenv.axon_hooks"] = mod
    antenv.axon_hooks = mod


def run(inputs, trace=False, tmpdir=None):
    """Run the kernel on all 8 cores; returns (full_output, BassKernelResults)."""
    if trace:
        _install_ntff_hook()
    nc = _get_program()
    in_maps = _prepare_inputs(inputs)
    res = run_bass_kernel_spmd(
        nc, in_maps, core_ids=list(range(NCORES)), trace=trace, tmpdir=tmpdir
    )
    # device emits tanh(x)+... transposed [OUT_COLS, BL] in bf16; the *0.1
    # scale and the transpose back to [BL, OUT_COLS] happen here.
    parts = []
    for i in range(NCORES):
        o = res.results[i]["outT"].astype(np.float32)
        parts.append(o.T * np.float32(STRENGTH))
    out = np.ascontiguousarray(np.concatenate(parts, axis=0),
                               dtype=np.float32)
    return out, res


def kernel(**inputs) -> np.ndarray:
    out, _ = run(inputs, trace=False)
    return out
